# revision 24
# baseline (speedup 1.0000x reference)
"""Trainium2 Bass kernel: nested-window average-pool deviation extractor.

Computes, for k in (7, 15, 31):  avg_pool_same_k(x) - x  (TF 'SAME' padding,
padded cells excluded from the average), stacked over k.

Input : [16, 512, 512, 3] f32   Output: [16, 3, 512, 512, 3] f32

Strategy (pure data-parallel over 8 cores, 2 images/core):
  * Horizontal pass (DVE): zero-padded per-channel prefix scan along W
    (tensor_tensor_scan), then one subtract per window size
    (box sum = P[j+r] - P[j-r-1]), written out as fp16.  Per-column count
    correction (k/cw(j), != 1 only near the left/right edge) applied to the
    edge columns with two tiny multiplies.
  * Vertical pass (PE): banded-matrix matmuls over the partition (H) axis,
    fp16 inputs, fp32 PSUM accumulation.  The 1/(ch(m)*k) normalization is
    folded into the band weights and the final "- x" is folded in as a -I
    matmul on the same accumulation group.
  * PSUM -> SBUF eviction on ScalarE, SBUF -> DRAM on HWDGE.
"""

import numpy as np

H = 512
W = 512
C = 3
B_TOTAL = 16
N_CORES = 8
B_CORE = B_TOTAL // N_CORES  # 2 images per core
NT = 4                        # 512 rows / 128 partitions
P = 128
FC = W * C                    # 1536
PAD = 16                      # prefix-buffer margin (pixels), >= r+1
XW = (W + 2 * PAD) * C        # 1632 prefix-buffer width
KS = (7, 15, 31)
EDGE = 16                     # edge columns carrying a count correction

_CACHE = {}


def _counts_1d(k):
    r = (k - 1) // 2
    idx = np.arange(W)
    return np.minimum(idx + r, W - 1) - np.maximum(idx - r, 0) + 1


def _weight_blocks():
    """All PE weight blocks, [nblk, 128, 128] fp16, plus index helpers.

    Layout: diag blocks idx s*4+t (s scale, t Mtile); 'up' blocks (K-tile t-1
    feeding M-tile t) at 12 + s*3 + (t-1); 'dn' blocks (K-tile t+1) at
    21 + s*3 + t; -I at 30.
    """
    blocks = np.zeros((31, P, P), np.float32)

    def band_block(k, tk, tm):
        r = (k - 1) // 2
        gk = 128 * tk + np.arange(P)[:, None]
        gm = 128 * tm + np.arange(P)[None, :]
        ch = _counts_1d(k)[None, 128 * tm: 128 * tm + P]
        return np.where(np.abs(gk - gm) <= r, 1.0 / (ch * k), 0.0)

    for s, k in enumerate(KS):
        for t in range(NT):
            blocks[s * 4 + t] = band_block(k, t, t)
        for t in range(1, NT):
            blocks[12 + s * 3 + (t - 1)] = band_block(k, t - 1, t)
        for t in range(NT - 1):
            blocks[21 + s * 3 + t] = band_block(k, t + 1, t)
    blocks[30] = -np.eye(P, dtype=np.float32)
    # host layout [P, nblk*P] so the SBUF load is a single contiguous DMA
    return np.ascontiguousarray(
        blocks.transpose(1, 0, 2).reshape(P, 31 * P)
    ).astype(np.float16)


def _edge_factors():
    """[P, 3*96+96] fp16: per scale, k/cw(j) for the 16 left + 16 right pixel
    columns (x3 channels interleaved); last 96 entries are zeros used to
    seed the x-tile margins."""
    out = np.ones((3, 96), np.float32)
    for s, k in enumerate(KS):
        cw = _counts_1d(k)
        fac = k / cw
        out[s, 0:48] = np.repeat(fac[0:EDGE], C)
        out[s, 48:96] = np.repeat(fac[W - EDGE:W], C)
    flat = np.concatenate([out.reshape(3 * 96), np.zeros(96, np.float32)])
    return np.broadcast_to(
        flat.reshape(1, 3 * 96 + 96), (P, 3 * 96 + 96)
    ).astype(np.float16).copy()


def _build_nc():
    import concourse.bacc as bacc
    import concourse.mybir as mybir
    import concourse.tile as tile
    from contextlib import ExitStack

    f16 = mybir.dt.float16
    f32 = mybir.dt.float32

    nc = bacc.Bacc("TRN2", target_bir_lowering=False, debug=False)
    x16 = nc.dram_tensor("x16", [B_CORE, NT, P, FC], f16, kind="ExternalInput")
    wblk = nc.dram_tensor("wblk", [P, 31 * P], f16, kind="ExternalInput")
    efac = nc.dram_tensor("efac", [P, 3 * 96 + 96], f16, kind="ExternalInput")
    y = nc.dram_tensor("y", [B_CORE, 3, NT, P, FC], f32, kind="ExternalOutput")

    ADD = mybir.AluOpType.add
    BYP = mybir.AluOpType.bypass

    with ExitStack() as ctx:
        tc = ctx.enter_context(tile.TileContext(nc))
        wpool = ctx.enter_context(tc.tile_pool(name="w", bufs=1))
        xpool = ctx.enter_context(tc.tile_pool(name="x", bufs=2 * NT))
        ppool = ctx.enter_context(tc.tile_pool(name="p", bufs=1))
        hpool = ctx.enter_context(tc.tile_pool(name="h", bufs=2 * NT))
        opool = ctx.enter_context(tc.tile_pool(name="o", bufs=6))
        pspool = ctx.enter_context(tc.tile_pool(name="ps", bufs=2, space="PSUM"))

        wt = wpool.tile([P, 31 * P], f16)
        nc.sync.dma_start(wt[:], wblk[:])
        et = wpool.tile([P, 3 * 96 + 96], f16)
        nc.sync.dma_start(et[:], efac[:])

        def wb(i):  # weight block i as a [128,128] lhsT
            return wt[:, P * i:P * (i + 1)]

        # persistent prefix buffers (one per unit), left margins zeroed once
        NU = B_CORE * NT
        Pbufs = [
            ppool.tile([P, XW], f32, tag=f"p{i}", name=f"pbuf{i}")
            for i in range(NU)
        ]
        for pb in Pbufs:
            nc.vector.tensor_copy(pb[:, 0:PAD * C], et[:, 3 * 96:3 * 96 + 48])

        hs = {}
        xs = {}
        for b in range(B_CORE):
            # ---- horizontal pass: per-channel prefix scan, then one
            # difference per window size; k=31 (and alternating k=15)
            # differences ride on the otherwise-idle GPSIMD ----
            for t in range(NT):
                u = b * NT + t
                X = xpool.tile([P, FC], f16, tag="x")
                nc.sync.dma_start(X[:], x16[b, t])
                Pt = Pbufs[u]
                Xv = X[:].rearrange("p (w c) -> p w c", c=C)
                Pv = Pt[:].rearrange("p (w c) -> p w c", c=C)
                for c in range(C):
                    nc.vector.tensor_tensor_scan(
                        Pv[:, PAD:PAD + W, c], Xv[:, :, c], Xv[:, :, c],
                        0.0, ADD, BYP,
                    )
                for c in range(C):
                    last = (PAD + W - 1) * C + c
                    nc.vector.tensor_copy(
                        Pv[:, PAD + W:PAD + W + PAD, c],
                        Pt[:, last:last + 1].broadcast_to([P, PAD]),
                    )
                for s, k in enumerate(KS):
                    r = (k - 1) // 2
                    hk = hpool.tile([P, FC], f16, tag=f"h{k}")
                    eng = nc.vector if (k == 7 or (k == 15 and u % 2)) \
                        else nc.gpsimd
                    eng.tensor_sub(
                        hk[:],
                        Pt[:, (PAD + r) * C:(PAD + r) * C + FC],
                        Pt[:, (PAD - r - 1) * C:(PAD - r - 1) * C + FC],
                    )
                    eng.tensor_mul(
                        hk[:, 0:48], hk[:, 0:48], et[:, 96 * s:96 * s + 48]
                    )
                    eng.tensor_mul(
                        hk[:, FC - 48:FC], hk[:, FC - 48:FC],
                        et[:, 96 * s + 48:96 * s + 96],
                    )
                    hs[(b, t, s)] = hk
                xs[(b, t)] = X

            # ---- vertical pass ----
            # weights-outer order: one LDWEIGHTS per block per group, three
            # N-slices streamed per load.  Cross-tile halo contributions are
            # 32x32 corner triangles packed via tile_position so they overlap
            # in the PE array.
            for s in range(len(KS)):
                for m in range(NT):
                    ps = pspool.tile([P, FC], f32, tag="ps")
                    SL = [slice(512 * n, 512 * (n + 1)) for n in range(3)]
                    for sl in SL:
                        nc.tensor.matmul(
                            ps[:, sl], wb(s * 4 + m), hs[(b, m, s)][:, sl],
                            start=True, stop=False,
                        )
                    if m > 0:
                        wu = wt[96:128, P * (12 + s * 3 + m - 1):
                                P * (12 + s * 3 + m - 1) + 32]
                        for sl in SL:
                            nc.tensor.matmul(
                                ps[0:32, sl], wu,
                                hs[(b, m - 1, s)][96:128, sl],
                                start=False, stop=False, tile_position=(96, 0),
                            )
                    if m < NT - 1:
                        wd = wt[0:32, P * (21 + s * 3 + m) + 96:
                                P * (21 + s * 3 + m) + 128]
                        for sl in SL:
                            nc.tensor.matmul(
                                ps[96:128, sl], wd,
                                hs[(b, m + 1, s)][0:32, sl],
                                start=False, stop=False, tile_position=(0, 96),
                            )
                    for sl in SL:
                        nc.tensor.matmul(
                            ps[:, sl], wb(30), xs[(b, m)][:, sl],
                            start=False, stop=True,
                        )
                    o = opool.tile([P, FC], f32, tag="o")
                    nc.scalar.copy(o[:], ps[:])
                    nc.sync.dma_start(y[b, s, m], o[:])
    nc.compile()
    return nc


def _get_compiled():
    if "nc" not in _CACHE:
        _CACHE["nc"] = _build_nc()
        _CACHE["wblk"] = _weight_blocks()
        _CACHE["efac"] = _edge_factors()
    return _CACHE["nc"], _CACHE["wblk"], _CACHE["efac"]


def kernel(inputs: np.ndarray) -> np.ndarray:
    from concourse.bass_utils import run_bass_kernel_spmd

    nc, wblk, efac = _get_compiled()
    x = np.asarray(inputs)
    assert x.shape == (B_TOTAL, H, W, C), x.shape
    x16 = x.astype(np.float16).reshape(N_CORES, B_CORE, NT, P, FC)
    in_maps = [
        {"x16": np.ascontiguousarray(x16[i]), "wblk": wblk, "efac": efac}
        for i in range(N_CORES)
    ]
    res = run_bass_kernel_spmd(nc, in_maps, core_ids=list(range(N_CORES)))
    outs = [
        r["y"].reshape(B_CORE, 3, H, W, C) for r in res.results
    ]
    return np.concatenate(outs, axis=0)


# revision 27
# speedup vs baseline: 1.1688x; 1.1688x over previous
"""Trainium2 Bass kernel: nested-window average-pool deviation extractor.

Computes, for k in (7, 15, 31):  avg_pool_same_k(x) - x  (TF 'SAME' padding,
padded cells excluded from the average), stacked over k.

Input : [16, 512, 512, 3] f32   Output: [16, 3, 512, 512, 3] f32

Strategy (pure data-parallel over 8 cores, 2 images/core):
  * Horizontal pass (DVE): zero-padded per-channel prefix scan along W
    (tensor_tensor_scan), then one subtract per window size
    (box sum = P[j+r] - P[j-r-1]), written out as fp16.  Per-column count
    correction (k/cw(j), != 1 only near the left/right edge) applied to the
    edge columns with two tiny multiplies.
  * Vertical pass (PE): banded-matrix matmuls over the partition (H) axis,
    fp16 inputs, fp32 PSUM accumulation.  The 1/(ch(m)*k) normalization is
    folded into the band weights and the final "- x" is folded in as a -I
    matmul on the same accumulation group.
  * PSUM -> SBUF eviction on ScalarE, SBUF -> DRAM on HWDGE.
"""

import numpy as np

H = 512
W = 512
C = 3
B_TOTAL = 16
N_CORES = 8
B_CORE = B_TOTAL // N_CORES  # 2 images per core
NT = 4                        # 512 rows / 128 partitions
P = 128
FC = W * C                    # 1536
PAD = 16                      # prefix-buffer margin (pixels), >= r+1
XW = (W + 2 * PAD) * C        # 1632 prefix-buffer width
KS = (7, 15, 31)
EDGE = 16                     # edge columns carrying a count correction

_CACHE = {}


def _counts_1d(k):
    r = (k - 1) // 2
    idx = np.arange(W)
    return np.minimum(idx + r, W - 1) - np.maximum(idx - r, 0) + 1


def _weight_blocks():
    """All PE weight blocks, [nblk, 128, 128] fp16, plus index helpers.

    Layout: diag blocks idx s*4+t (s scale, t Mtile); 'up' blocks (K-tile t-1
    feeding M-tile t) at 12 + s*3 + (t-1); 'dn' blocks (K-tile t+1) at
    21 + s*3 + t; -I at 30.
    """
    blocks = np.zeros((31, P, P), np.float32)

    def band_block(k, tk, tm):
        r = (k - 1) // 2
        gk = 128 * tk + np.arange(P)[:, None]
        gm = 128 * tm + np.arange(P)[None, :]
        ch = _counts_1d(k)[None, 128 * tm: 128 * tm + P]
        return np.where(np.abs(gk - gm) <= r, 1.0 / (ch * k), 0.0)

    for s, k in enumerate(KS):
        for t in range(NT):
            blocks[s * 4 + t] = band_block(k, t, t)
        for t in range(1, NT):
            blocks[12 + s * 3 + (t - 1)] = band_block(k, t - 1, t)
        for t in range(NT - 1):
            blocks[21 + s * 3 + t] = band_block(k, t + 1, t)
    blocks[30] = -np.eye(P, dtype=np.float32)
    # host layout [P, nblk*P] so the SBUF load is a single contiguous DMA
    return np.ascontiguousarray(
        blocks.transpose(1, 0, 2).reshape(P, 31 * P)
    ).astype(np.float16)


def _edge_factors():
    """[P, 3*96+96] fp16: per scale, k/cw(j) for the 16 left + 16 right pixel
    columns (x3 channels interleaved); last 96 entries are zeros used to
    seed the x-tile margins."""
    out = np.ones((3, 96), np.float32)
    for s, k in enumerate(KS):
        cw = _counts_1d(k)
        fac = k / cw
        out[s, 0:48] = np.repeat(fac[0:EDGE], C)
        out[s, 48:96] = np.repeat(fac[W - EDGE:W], C)
    flat = np.concatenate([out.reshape(3 * 96), np.zeros(96, np.float32)])
    return np.broadcast_to(
        flat.reshape(1, 3 * 96 + 96), (P, 3 * 96 + 96)
    ).astype(np.float16).copy()


def _build_nc():
    import concourse.bacc as bacc
    import concourse.mybir as mybir
    import concourse.tile as tile
    from contextlib import ExitStack

    f16 = mybir.dt.float16
    f32 = mybir.dt.float32

    nc = bacc.Bacc("TRN2", target_bir_lowering=False, debug=False)
    x16 = nc.dram_tensor("x16", [B_CORE, NT, P, FC], f16, kind="ExternalInput")
    wblk = nc.dram_tensor("wblk", [P, 31 * P], f16, kind="ExternalInput")
    efac = nc.dram_tensor("efac", [P, 3 * 96 + 96], f16, kind="ExternalInput")
    y = nc.dram_tensor("y", [B_CORE, 3, NT, P, FC], f32, kind="ExternalOutput")

    ADD = mybir.AluOpType.add
    BYP = mybir.AluOpType.bypass

    with ExitStack() as ctx:
        tc = ctx.enter_context(tile.TileContext(nc))
        wpool = ctx.enter_context(tc.tile_pool(name="w", bufs=1))
        xpool = ctx.enter_context(tc.tile_pool(name="x", bufs=2 * NT))
        ppool = ctx.enter_context(tc.tile_pool(name="p", bufs=1))
        hpool = ctx.enter_context(tc.tile_pool(name="h", bufs=2 * NT))
        opool = ctx.enter_context(tc.tile_pool(name="o", bufs=4))
        pspool = ctx.enter_context(tc.tile_pool(name="ps", bufs=2, space="PSUM"))

        wt = wpool.tile([P, 31 * P], f16)
        nc.sync.dma_start(wt[:], wblk[:])
        et = wpool.tile([P, 3 * 96 + 96], f16)
        nc.sync.dma_start(et[:], efac[:])

        def wb(i):  # weight block i as a [128,128] lhsT
            return wt[:, P * i:P * (i + 1)]

        # persistent prefix buffers (one per unit), left margins zeroed once
        NU = B_CORE * NT
        Pbufs = [
            ppool.tile([P, XW], f32, tag=f"p{i}", name=f"pbuf{i}")
            for i in range(NU)
        ]
        for pb in Pbufs:
            nc.vector.tensor_copy(pb[:, 0:PAD * C], et[:, 3 * 96:3 * 96 + 48])

        hs = {}
        xs = {}
        for b in range(B_CORE):
            # ---- horizontal pass: per-channel prefix scan, then one
            # difference per window size; k=31 (and alternating k=15)
            # differences ride on the otherwise-idle GPSIMD ----
            for t in range(NT):
                u = b * NT + t
                X = xpool.tile([P, FC], f16, tag="x")
                nc.sync.dma_start(X[:], x16[b, t])
                Pt = Pbufs[u]
                Xv = X[:].rearrange("p (w c) -> p w c", c=C)
                Pv = Pt[:].rearrange("p (w c) -> p w c", c=C)
                for c in range(C):
                    nc.vector.tensor_tensor_scan(
                        Pv[:, PAD:PAD + W, c], Xv[:, :, c], Xv[:, :, c],
                        0.0, ADD, BYP,
                    )
                for c in range(C):
                    last = (PAD + W - 1) * C + c
                    nc.vector.tensor_copy(
                        Pv[:, PAD + W:PAD + W + PAD, c],
                        Pt[:, last:last + 1].broadcast_to([P, PAD]),
                    )
                for s, k in enumerate(KS):
                    r = (k - 1) // 2
                    hk = hpool.tile([P, FC], f16, tag=f"h{k}")
                    eng = nc.vector
                    eng.tensor_sub(
                        hk[:],
                        Pt[:, (PAD + r) * C:(PAD + r) * C + FC],
                        Pt[:, (PAD - r - 1) * C:(PAD - r - 1) * C + FC],
                    )
                    eng.tensor_mul(
                        hk[:, 0:48], hk[:, 0:48], et[:, 96 * s:96 * s + 48]
                    )
                    eng.tensor_mul(
                        hk[:, FC - 48:FC], hk[:, FC - 48:FC],
                        et[:, 96 * s + 48:96 * s + 96],
                    )
                    hs[(b, t, s)] = hk
                xs[(b, t)] = X

            # ---- vertical pass ----
            # weights-outer order: one LDWEIGHTS per block per group, three
            # N-slices streamed per load.  Cross-tile halo contributions are
            # 32x32 corner triangles packed via tile_position so they overlap
            # in the PE array.
            for s in range(len(KS)):
                for m in range(NT):
                    ps = pspool.tile([P, FC], f32, tag="ps")
                    for n in range(3):
                        sl = slice(512 * n, 512 * (n + 1))
                        nc.tensor.matmul(
                            ps[:, sl], wb(s * 4 + m), hs[(b, m, s)][:, sl],
                            start=True, stop=False,
                        )
                        if m > 0:
                            nc.tensor.matmul(
                                ps[:, sl], wb(12 + s * 3 + (m - 1)),
                                hs[(b, m - 1, s)][:, sl],
                                start=False, stop=False,
                            )
                        if m < NT - 1:
                            nc.tensor.matmul(
                                ps[:, sl], wb(21 + s * 3 + m),
                                hs[(b, m + 1, s)][:, sl],
                                start=False, stop=False,
                            )
                        nc.tensor.matmul(
                            ps[:, sl], wb(30), xs[(b, m)][:, sl],
                            start=False, stop=True,
                        )
                    o = opool.tile([P, FC], f32, tag="o")
                    nc.scalar.copy(o[:], ps[:])
                    nc.sync.dma_start(y[b, s, m], o[:])
    nc.compile()
    return nc


def _get_compiled():
    if "nc" not in _CACHE:
        _CACHE["nc"] = _build_nc()
        _CACHE["wblk"] = _weight_blocks()
        _CACHE["efac"] = _edge_factors()
    return _CACHE["nc"], _CACHE["wblk"], _CACHE["efac"]


def kernel(inputs: np.ndarray) -> np.ndarray:
    from concourse.bass_utils import run_bass_kernel_spmd

    nc, wblk, efac = _get_compiled()
    x = np.asarray(inputs)
    assert x.shape == (B_TOTAL, H, W, C), x.shape
    x16 = x.astype(np.float16).reshape(N_CORES, B_CORE, NT, P, FC)
    in_maps = [
        {"x16": np.ascontiguousarray(x16[i]), "wblk": wblk, "efac": efac}
        for i in range(N_CORES)
    ]
    res = run_bass_kernel_spmd(nc, in_maps, core_ids=list(range(N_CORES)))
    outs = [
        r["y"].reshape(B_CORE, 3, H, W, C) for r in res.results
    ]
    return np.concatenate(outs, axis=0)


# revision 28
# speedup vs baseline: 1.3063x; 1.1177x over previous
"""Trainium2 Bass kernel: nested-window average-pool deviation extractor.

Computes, for k in (7, 15, 31):  avg_pool_same_k(x) - x  (TF 'SAME' padding,
padded cells excluded from the average), stacked over k.

Input : [16, 512, 512, 3] f32   Output: [16, 3, 512, 512, 3] f32

Strategy (pure data-parallel over 8 cores, 2 images/core):
  * Horizontal pass (DVE): zero-padded per-channel prefix scan along W
    (tensor_tensor_scan), then one subtract per window size
    (box sum = P[j+r] - P[j-r-1]), written out as fp16.  Per-column count
    correction (k/cw(j), != 1 only near the left/right edge) applied to the
    edge columns with two tiny multiplies.
  * Vertical pass (PE): banded-matrix matmuls over the partition (H) axis,
    fp16 inputs, fp32 PSUM accumulation.  The 1/(ch(m)*k) normalization is
    folded into the band weights and the final "- x" is folded in as a -I
    matmul on the same accumulation group.
  * PSUM -> SBUF eviction on ScalarE, SBUF -> DRAM on HWDGE.
"""

import numpy as np

H = 512
W = 512
C = 3
B_TOTAL = 16
N_CORES = 8
B_CORE = B_TOTAL // N_CORES  # 2 images per core
NT = 4                        # 512 rows / 128 partitions
P = 128
FC = W * C                    # 1536
PAD = 16                      # prefix-buffer margin (pixels), >= r+1
XW = (W + 2 * PAD) * C        # 1632 prefix-buffer width
KS = (7, 15, 31)
EDGE = 16                     # edge columns carrying a count correction

_CACHE = {}


def _counts_1d(k):
    r = (k - 1) // 2
    idx = np.arange(W)
    return np.minimum(idx + r, W - 1) - np.maximum(idx - r, 0) + 1


def _weight_blocks():
    """All PE weight blocks, [nblk, 128, 128] fp16, plus index helpers.

    Layout: diag blocks idx s*4+t (s scale, t Mtile); 'up' blocks (K-tile t-1
    feeding M-tile t) at 12 + s*3 + (t-1); 'dn' blocks (K-tile t+1) at
    21 + s*3 + t; -I at 30.
    """
    blocks = np.zeros((31, P, P), np.float32)

    def band_block(k, tk, tm):
        r = (k - 1) // 2
        gk = 128 * tk + np.arange(P)[:, None]
        gm = 128 * tm + np.arange(P)[None, :]
        ch = _counts_1d(k)[None, 128 * tm: 128 * tm + P]
        return np.where(np.abs(gk - gm) <= r, 1.0 / (ch * k), 0.0)

    for s, k in enumerate(KS):
        for t in range(NT):
            blocks[s * 4 + t] = band_block(k, t, t)
        for t in range(1, NT):
            blocks[12 + s * 3 + (t - 1)] = band_block(k, t - 1, t)
        for t in range(NT - 1):
            blocks[21 + s * 3 + t] = band_block(k, t + 1, t)
    blocks[30] = -np.eye(P, dtype=np.float32)
    # host layout [P, nblk*P] so the SBUF load is a single contiguous DMA
    return np.ascontiguousarray(
        blocks.transpose(1, 0, 2).reshape(P, 31 * P)
    ).astype(np.float16)


def _edge_factors():
    """[P, 3*96+96] fp16: per scale, k/cw(j) for the 16 left + 16 right pixel
    columns (x3 channels interleaved); last 96 entries are zeros used to
    seed the x-tile margins."""
    out = np.ones((3, 96), np.float32)
    for s, k in enumerate(KS):
        cw = _counts_1d(k)
        fac = k / cw
        out[s, 0:48] = np.repeat(fac[0:EDGE], C)
        out[s, 48:96] = np.repeat(fac[W - EDGE:W], C)
    flat = np.concatenate([out.reshape(3 * 96), np.zeros(96, np.float32)])
    return np.broadcast_to(
        flat.reshape(1, 3 * 96 + 96), (P, 3 * 96 + 96)
    ).astype(np.float16).copy()


def _build_nc():
    import concourse.bacc as bacc
    import concourse.mybir as mybir
    import concourse.tile as tile
    from contextlib import ExitStack

    f16 = mybir.dt.float16
    f32 = mybir.dt.float32

    nc = bacc.Bacc("TRN2", target_bir_lowering=False, debug=False)
    x16 = nc.dram_tensor("x16", [B_CORE, NT, P, FC], f16, kind="ExternalInput")
    wblk = nc.dram_tensor("wblk", [P, 31 * P], f16, kind="ExternalInput")
    efac = nc.dram_tensor("efac", [P, 3 * 96 + 96], f16, kind="ExternalInput")
    y = nc.dram_tensor("y", [B_CORE, 3, NT, P, FC], f32, kind="ExternalOutput")

    ADD = mybir.AluOpType.add
    BYP = mybir.AluOpType.bypass

    with ExitStack() as ctx:
        tc = ctx.enter_context(tile.TileContext(nc))
        wpool = ctx.enter_context(tc.tile_pool(name="w", bufs=1))
        xpool = ctx.enter_context(tc.tile_pool(name="x", bufs=2 * NT))
        ppool = ctx.enter_context(tc.tile_pool(name="p", bufs=1))
        hpool = ctx.enter_context(tc.tile_pool(name="h", bufs=2 * NT))
        opool = ctx.enter_context(tc.tile_pool(name="o", bufs=4))
        pspool = ctx.enter_context(tc.tile_pool(name="ps", bufs=2, space="PSUM"))

        wt = wpool.tile([P, 31 * P], f16)
        nc.sync.dma_start(wt[:], wblk[:])
        et = wpool.tile([P, 3 * 96 + 96], f16)
        nc.sync.dma_start(et[:], efac[:])

        def wb(i):  # weight block i as a [128,128] lhsT
            return wt[:, P * i:P * (i + 1)]

        # persistent prefix buffers (one per unit), left margins zeroed once
        NU = B_CORE * NT
        Pbufs = [
            ppool.tile([P, XW], f32, tag=f"p{i}", name=f"pbuf{i}")
            for i in range(NU)
        ]
        for pb in Pbufs:
            nc.vector.tensor_copy(pb[:, 0:PAD * C], et[:, 3 * 96:3 * 96 + 48])

        hs = {}
        xs = {}
        for b in range(B_CORE):
            # ---- horizontal pass: per-channel prefix scan, then one
            # difference per window size; k=31 (and alternating k=15)
            # differences ride on the otherwise-idle GPSIMD ----
            for t in range(NT):
                u = b * NT + t
                X = xpool.tile([P, FC], f16, tag="x")
                nc.sync.dma_start(X[:], x16[b, t])
                Pt = Pbufs[u]
                Xv = X[:].rearrange("p (w c) -> p w c", c=C)
                Pv = Pt[:].rearrange("p (w c) -> p w c", c=C)
                for c in range(C):
                    nc.vector.tensor_tensor_scan(
                        Pv[:, PAD:PAD + W, c], Xv[:, :, c], Xv[:, :, c],
                        0.0, ADD, BYP,
                    )
                for c in range(C):
                    last = (PAD + W - 1) * C + c
                    nc.vector.tensor_copy(
                        Pv[:, PAD + W:PAD + W + PAD, c],
                        Pt[:, last:last + 1].broadcast_to([P, PAD]),
                    )
                for s, k in enumerate(KS):
                    r = (k - 1) // 2
                    hk = hpool.tile([P, FC], f16, tag=f"h{k}")
                    eng = nc.vector
                    eng.tensor_sub(
                        hk[:],
                        Pt[:, (PAD + r) * C:(PAD + r) * C + FC],
                        Pt[:, (PAD - r - 1) * C:(PAD - r - 1) * C + FC],
                    )
                    eng.tensor_mul(
                        hk[:, 0:48], hk[:, 0:48], et[:, 96 * s:96 * s + 48]
                    )
                    eng.tensor_mul(
                        hk[:, FC - 48:FC], hk[:, FC - 48:FC],
                        et[:, 96 * s + 48:96 * s + 96],
                    )
                    hs[(b, t, s)] = hk
                xs[(b, t)] = X

            # ---- vertical pass ----
            # weights-outer order: one LDWEIGHTS per block per group, three
            # N-slices streamed per load.  Cross-tile halo contributions are
            # 32x32 corner triangles packed via tile_position so they overlap
            # in the PE array.
            for m in range(NT):
                for s in range(len(KS)):
                    ps = pspool.tile([P, FC], f32, tag="ps")
                    for n in range(3):
                        sl = slice(512 * n, 512 * (n + 1))
                        nc.tensor.matmul(
                            ps[:, sl], wb(s * 4 + m), hs[(b, m, s)][:, sl],
                            start=True, stop=False,
                        )
                        if m > 0:
                            nc.tensor.matmul(
                                ps[:, sl], wb(12 + s * 3 + (m - 1)),
                                hs[(b, m - 1, s)][:, sl],
                                start=False, stop=False,
                            )
                        if m < NT - 1:
                            nc.tensor.matmul(
                                ps[:, sl], wb(21 + s * 3 + m),
                                hs[(b, m + 1, s)][:, sl],
                                start=False, stop=False,
                            )
                        nc.tensor.matmul(
                            ps[:, sl], wb(30), xs[(b, m)][:, sl],
                            start=False, stop=True,
                        )
                    o = opool.tile([P, FC], f32, tag="o")
                    nc.scalar.copy(o[:], ps[:])
                    nc.sync.dma_start(y[b, s, m], o[:])
    nc.compile()
    return nc


def _get_compiled():
    if "nc" not in _CACHE:
        _CACHE["nc"] = _build_nc()
        _CACHE["wblk"] = _weight_blocks()
        _CACHE["efac"] = _edge_factors()
    return _CACHE["nc"], _CACHE["wblk"], _CACHE["efac"]


def kernel(inputs: np.ndarray) -> np.ndarray:
    from concourse.bass_utils import run_bass_kernel_spmd

    nc, wblk, efac = _get_compiled()
    x = np.asarray(inputs)
    assert x.shape == (B_TOTAL, H, W, C), x.shape
    x16 = x.astype(np.float16).reshape(N_CORES, B_CORE, NT, P, FC)
    in_maps = [
        {"x16": np.ascontiguousarray(x16[i]), "wblk": wblk, "efac": efac}
        for i in range(N_CORES)
    ]
    res = run_bass_kernel_spmd(nc, in_maps, core_ids=list(range(N_CORES)))
    outs = [
        r["y"].reshape(B_CORE, 3, H, W, C) for r in res.results
    ]
    return np.concatenate(outs, axis=0)
